# revision 16
# baseline (speedup 1.0000x reference)
"""Trainium2 Bass kernel for nn_DSP_33131377721365 (v3).

reference math (x: [4, 32, 720, 720] f32, conv_w: [32, 32, 3, 1] f32):
  s[b,h,w]    = sum_c x[b,c,h,w]
  d[b,h,w]    = (1/9) * sum_{t=0..8} s[b, h+t-4, w+t-4]   (zero padded)
  out[b,o,h,w]= sum_{j=0..2} wsum[o,j] * d[b, h-1+j, w]   (zero padded)
  where wsum[o,j] = sum_c conv_w[o,c,j,0]

Sharding: 8 cores = 4 batches x 2 H-halves (360 output rows each). All HBM
I/O bf16 (host casts).

v3 changes vs v2 (both measured on the axon-tunneled cores):
 * Input rides 3 disjoint [128, 23040] row-aligned DMAs (shard zero-padded
   to 384 rows). 128-partition transfers run ~3x faster than the v2
   [100, C, W] tiles (616 vs 191 GB/s measured): partition count must be
   128 to engage all 16 SDMA engines evenly.
 * Channel-sum tree runs per 128-row chunk (full partitions) in W-halves.
 * Stage-1 9-tap diagonal pool: per (block, tap) the 100-row s-window may
   straddle two chunks; bands are [128, 92] per (block, tap, chunk) and
   the matmuls accumulate across chunk parts in PSUM. K=128 from partition
   0 dodges the engine-AP base%32 rule.
 * Stage-2 packs output rows STRIDED by 3: sub-block s = out rows
   {s, s+3, ..., s+87} (30 rows x 4 channels = M 120). Every sub-block then
   reads d[0:92] from partition 0, so stage-1 PSUM evacuates with ONE copy
   per W-half into a single [92, 720] d tile (v2 needed 4 copies into 3
   window tiles), and all stage-2 rhs APs are legal.
Host reassembles/casts the bf16 output back to f32 (un-striding rows).
"""

import numpy as np
import ml_dtypes

import concourse.bass as bass
import concourse.bacc as bacc
import concourse.mybir as mybir
import concourse.tile as tile
from concourse.bass_utils import run_bass_kernel_spmd

FP = mybir.dt.float32
BF = mybir.dt.bfloat16
NPBF = ml_dtypes.bfloat16

B, C, H, W = 4, 32, 720, 720
O = 32
N_CORES = 8
HS = H // 2          # 360 output rows per core
BLK = 90             # output rows per block
NBLK = HS // BLK     # 4
SHR = 370            # shard rows; 3 chunks of 128 (last end-aligned)
NCH = 3              # chunks per core
ROW = C * W          # 23040 elems per shard row
M1 = BLK + 2         # 92 d rows per block
NS = 30              # rows per stage-2 sub-block (strided by 3)
OSUB = 4             # output channels per stage-2 matmul group
NOG = O // OSUB      # 8 o-groups
M2 = OSUB * NS       # 120
WPAD = 4
SPW = W + 2 * WPAD   # 728
KTAPS = 9
HALO = 5

# stage-2 evacuation engine split: (s, og-pair) combos handled by DVE (rest
# ACT). Balance: DVE has the channel-sum tree (~37us/rep), ACT the stage-1
# evac; cost-model rates: ACT 1.385us, DVE 1.63us per stage-2 evac op.
# ~11 of 48 ops/rep on DVE equalizes both at ~57us/rep.
DVE_EVAC_OPS = ((0, 3), (1, 3), (2, 3))

# chunk start rows: disjoint except chunk 2, end-aligned to the 370-row
# shard (saves 14 zero-pad rows of HBM traffic). CLAIM = which shard rows
# each chunk OWNS for band construction (chunks 1/2 overlap in rows
# 242..256; chunk 1 owns them).
CH0 = (0, 128, SHR - 128)
CLAIM = ((0, 128), (128, 256), (256, SHR))


def _mm_parts(b):
    """[(tap, chunk)]: which chunks own rows that tap t of block b needs."""
    parts = []
    for t in range(KTAPS):
        lo, hi = 90 * b + t, 90 * b + t + M1  # shard rows [lo, hi)
        for c in range(NCH):
            clo, chi = CLAIM[c]
            if lo < chi and hi > clo:
                parts.append((t, c))
    return parts


def _build(nc, reps=1):
    xs = nc.declare_dram_parameter("xs", [SHR, ROW], BF, isOutput=False)
    nmm = sum(len(_mm_parts(b)) for b in range(NBLK))
    bd = nc.declare_dram_parameter("bands", [128, nmm, M1], BF, isOutput=False)
    am = nc.declare_dram_parameter("amc", [M1, 3, NOG, M2], BF, isOutput=False)
    # [blk, s, p, og, w]: one DMA per (block, sub-block); rows 120..128 junk
    out = nc.declare_dram_parameter("out", [NBLK, 3, 128, NOG, W], BF,
                                    isOutput=True)

    add = mybir.AluOpType.add

    with tile.TileContext(nc) as tc:
        with (
            tc.tile_pool(name="xa", bufs=3) as xpool,
            tc.tile_pool(name="tr", bufs=1) as tpool,
            tc.tile_pool(name="sp", bufs=3) as spool,
            tc.tile_pool(name="dd", bufs=4) as dpool,
            tc.tile_pool(name="ob", bufs=2) as opool,
            tc.tile_pool(name="cst", bufs=1) as cpool,
            tc.tile_pool(name="ps1", bufs=2, space="PSUM") as ps1pool,
            tc.tile_pool(name="ps2", bufs=2, space="PSUM") as ps2pool,
        ):
            bdt = cpool.tile([128, nmm, M1], BF)
            nc.sync.dma_start(bdt[:], bd[:])
            amt = cpool.tile([M1, 3, NOG, M2], BF)
            nc.sync.dma_start(amt[:], am[:])

            for it in range(reps):
                sps = []
                for ci in range(NCH):
                    xa = xpool.tile([128, ROW], BF, tag="xa", name=f"xa{ci}")
                    nc.sync.dma_start(xa[:], xs[CH0[ci]:CH0[ci] + 128])
                    xv = xa.rearrange("p (c w) -> p c w", c=C)
                    sp = spool.tile([128, SPW], BF, tag="sp", name=f"sp{ci}")
                    nc.vector.memset(sp[:, 0:WPAD], 0.0)
                    nc.vector.memset(sp[:, WPAD + W:SPW], 0.0)
                    # channel-sum tree in W-halves (bf16 2x DVE mode)
                    t1 = tpool.tile([128, 16, 360], BF, tag="t1", name="t1")
                    t2 = tpool.tile([128, 8, 360], BF, tag="t2", name="t2")
                    for h0 in (0, 360):
                        h1 = h0 + 360
                        nc.vector.tensor_tensor(out=t1[:],
                                                in0=xv[:, 0:16, h0:h1],
                                                in1=xv[:, 16:32, h0:h1],
                                                op=add)
                        nc.vector.tensor_tensor(out=t2[:], in0=t1[:, 0:8],
                                                in1=t1[:, 8:16], op=add)
                        t3 = t1[:, 0:4, :]
                        nc.vector.tensor_tensor(out=t3, in0=t2[:, 0:4],
                                                in1=t2[:, 4:8], op=add)
                        t4 = t2[:, 0:2, :]
                        nc.vector.tensor_tensor(out=t4, in0=t3[:, 0:2],
                                                in1=t3[:, 2:4], op=add)
                        nc.vector.tensor_tensor(
                            out=sp[:, WPAD + h0:WPAD + h1],
                            in0=t4[:, 0], in1=t4[:, 1], op=add)
                    sps.append(sp)

                # stage 1 for ALL blocks first: PE never stalls behind a
                # block's d-evac; d tiles (dd bufs=4) hold all four blocks
                dts = []
                mi = 0
                for blk in range(NBLK):
                    parts = _mm_parts(blk)
                    # 9-tap diagonal pool: accumulate over (tap, chunk) parts
                    dt = dpool.tile([M1, W], BF, tag="dt", name=f"dt{blk}")
                    for w0 in (0, 360):
                        ps1 = ps1pool.tile([M1, 360], FP, tag="ps1",
                                           name="ps1")
                        for pi, (t, ci) in enumerate(parts):
                            nc.tensor.matmul(
                                ps1[:], bdt[:, mi + pi, :],
                                sps[ci][:, w0 + t:w0 + t + 360],
                                start=(pi == 0), stop=(pi == len(parts) - 1),
                            )
                        nc.scalar.copy(out=dt[:, w0:w0 + 360], in_=ps1[:])
                    mi += len(parts)
                    dts.append(dt)

                for blk in range(NBLK):
                    dt = dts[blk]
                    # 3x1 conv + channel broadcast: strided-row packing,
                    # M = 4 channels x 30 rows; og pairs share a psum tile;
                    # per-s output DMA so the out ring drains early
                    for s in range(3):
                        obs = opool.tile([128, NOG, W], BF, tag="ob",
                                         name="obs")
                        for op in range(NOG // 2):
                            ps2 = ps2pool.tile([M2, 2 * W], FP, tag="ps2",
                                               name="ps2")
                            lhsA = amt[:, s, 2 * op, :]
                            lhsB = amt[:, s, 2 * op + 1, :]
                            nc.tensor.matmul(ps2[:, 0:512], lhsA,
                                             dt[0:M1, 0:512],
                                             start=True, stop=True)
                            nc.tensor.matmul(ps2[:, 512:720], lhsA,
                                             dt[0:M1, 512:W],
                                             start=True, stop=True)
                            nc.tensor.matmul(ps2[:, 720:1024], lhsB,
                                             dt[0:M1, 0:304],
                                             start=True, stop=True)
                            nc.tensor.matmul(ps2[:, 1024:1440], lhsB,
                                             dt[0:M1, 304:W],
                                             start=True, stop=True)
                            dst = obs[0:M2, 2 * op:2 * op + 2, :]
                            src = ps2[:].rearrange("p (g w) -> p g w", g=2)
                            if (s, op) in DVE_EVAC_OPS:
                                nc.vector.tensor_copy(out=dst, in_=src)
                            else:
                                nc.scalar.copy(out=dst, in_=src)
                        nc.scalar.dma_start(out[blk, s], obs[:])
    return nc


def _make_bands(half):
    """[128, nmm, 92] bf16 stage-1 bands, one [128, 92] slab per
    (block, tap, chunk) matmul, 1/9 scaled; d rows outside the global image
    zeroed (conv zero padding)."""
    nmm = sum(len(_mm_parts(b)) for b in range(NBLK))
    bands = np.zeros((128, nmm, M1), np.float32)
    mi = 0
    for b in range(NBLK):
        for (t, c) in _mm_parts(b):
            clo, chi = CLAIM[c]
            for m in range(M1):
                if half == 0 and b == 0 and m == 0:
                    continue  # d row -1
                if half == 1 and b == NBLK - 1 and m == M1 - 1:
                    continue  # d row 720
                r = 90 * b + m + t
                if clo <= r < chi:
                    bands[r - CH0[c], mi, m] = 1.0 / KTAPS
            mi += 1
    return bands.astype(NPBF)


def _make_amc(conv_w):
    """[92, 3, NOG, 120] bf16: 3-tap conv bands, strided-row packing.
    Sub-block s, slot oi*30+m -> out channel og*4+oi, block row 3m+s."""
    wsum = conv_w.sum(axis=1)[:, :, 0].astype(np.float64)  # [O, 3]
    amc = np.zeros((M1, 3, NOG, M2), np.float32)
    for s in range(3):
        for og in range(NOG):
            for oi in range(OSUB):
                o = og * OSUB + oi
                for m in range(NS):
                    for j in range(3):
                        amc[3 * m + s + j, s, og, oi * NS + m] = wsum[o, j]
    return amc.astype(NPBF)


def _make_shard(xt_b, h0):
    """xt_b: [H, ROW] bf16 one batch (h-major rows). [370, ROW] zero-pad."""
    sh = np.zeros((SHR, ROW), NPBF)
    lo, hi = h0 - HALO, h0 + HS + HALO
    slo, shi = max(lo, 0), min(hi, H)
    sh[slo - lo:shi - lo] = xt_b[slo:shi]
    return sh


def make_in_maps(x, conv_w):
    x = np.ascontiguousarray(np.asarray(x, dtype=np.float32))
    conv_w = np.asarray(conv_w, dtype=np.float32)
    assert x.shape == (B, C, H, W) and conv_w.shape == (O, C, 3, 1)
    xt = np.ascontiguousarray(x.transpose(0, 2, 1, 3)).astype(NPBF)
    xt = xt.reshape(B, H, ROW)
    amc = _make_amc(conv_w)
    bands = [_make_bands(0), _make_bands(1)]
    in_maps = []
    for i in range(N_CORES):
        b, half = i // 2, i % 2
        in_maps.append({
            "xs": _make_shard(xt[b], half * HS),
            "bands": bands[half],
            "amc": amc,
        })
    return in_maps


def assemble_out(results):
    out = np.empty((B, O, H, W), np.float32)
    for i in range(N_CORES):
        b, half = i // 2, i % 2
        v = np.asarray(results[i]["out"]).astype(np.float32)
        ov = np.empty((O, HS, W), np.float32)
        for blk in range(NBLK):
            for s in range(3):
                w = v[blk, s, 0:M2, :, :]             # [4*30, NOG, W]
                w = w.reshape(OSUB, NS, NOG, W)
                w = w.transpose(2, 0, 1, 3).reshape(O, NS, W)
                ov[:, blk * BLK + s:blk * BLK + 90:3, :] = w
        out[b, :, half * HS:(half + 1) * HS, :] = ov
    return out


def kernel(x, conv_w):
    nc = bacc.Bacc("TRN2", target_bir_lowering=False, debug=False,
                   num_devices=N_CORES)
    _build(nc)
    nc.compile()
    res = run_bass_kernel_spmd(nc, make_in_maps(x, conv_w),
                               list(range(N_CORES)), trace=False)
    return assemble_out(res.results)


# revision 17
# speedup vs baseline: 2.3476x; 2.3476x over previous
"""Trainium2 Bass kernel for nn_DSP_33131377721365 (v3).

reference math (x: [4, 32, 720, 720] f32, conv_w: [32, 32, 3, 1] f32):
  s[b,h,w]    = sum_c x[b,c,h,w]
  d[b,h,w]    = (1/9) * sum_{t=0..8} s[b, h+t-4, w+t-4]   (zero padded)
  out[b,o,h,w]= sum_{j=0..2} wsum[o,j] * d[b, h-1+j, w]   (zero padded)
  where wsum[o,j] = sum_c conv_w[o,c,j,0]

Sharding: 8 cores = 4 batches x 2 H-halves (360 output rows each). All HBM
I/O bf16 (host casts).

v3 changes vs v2 (both measured on the axon-tunneled cores):
 * Input rides 3 disjoint [128, 23040] row-aligned DMAs (shard zero-padded
   to 384 rows). 128-partition transfers run ~3x faster than the v2
   [100, C, W] tiles (616 vs 191 GB/s measured): partition count must be
   128 to engage all 16 SDMA engines evenly.
 * Channel-sum tree runs per 128-row chunk (full partitions) in W-halves.
 * Stage-1 9-tap diagonal pool: per (block, tap) the 100-row s-window may
   straddle two chunks; bands are [128, 92] per (block, tap, chunk) and
   the matmuls accumulate across chunk parts in PSUM. K=128 from partition
   0 dodges the engine-AP base%32 rule.
 * Stage-2 packs output rows STRIDED by 3: sub-block s = out rows
   {s, s+3, ..., s+87} (30 rows x 4 channels = M 120). Every sub-block then
   reads d[0:92] from partition 0, so stage-1 PSUM evacuates with ONE copy
   per W-half into a single [92, 720] d tile (v2 needed 4 copies into 3
   window tiles), and all stage-2 rhs APs are legal.
Host reassembles/casts the bf16 output back to f32 (un-striding rows).
"""

import numpy as np
import ml_dtypes

import concourse.bass as bass
import concourse.bacc as bacc
import concourse.mybir as mybir
import concourse.tile as tile
from concourse.bass_utils import run_bass_kernel_spmd

FP = mybir.dt.float32
BF = mybir.dt.bfloat16
NPBF = ml_dtypes.bfloat16

B, C, H, W = 4, 32, 720, 720
O = 32
N_CORES = 8
HS = H // 2          # 360 output rows per core
BLK = 90             # output rows per block
NBLK = HS // BLK     # 4
SHR = 370            # shard rows; 3 chunks of 128 (last end-aligned)
NCH = 3              # chunks per core
ROW = C * W          # 23040 elems per shard row
M1 = BLK + 2         # 92 d rows per block
NS = 30              # rows per stage-2 sub-block (strided by 3)
OSUB = 4             # output channels per stage-2 matmul group
NOG = O // OSUB      # 8 o-groups
M2 = OSUB * NS       # 120
WPAD = 4
SPW = W + 2 * WPAD   # 728
KTAPS = 9
HALO = 5

# stage-2 evacuation engine split: (s, og-pair) combos handled by DVE (rest
# ACT). Balance: DVE has the channel-sum tree (~37us/rep), ACT the stage-1
# evac; cost-model rates: ACT 1.385us, DVE 1.63us per stage-2 evac op.
# ~11 of 48 ops/rep on DVE equalizes both at ~57us/rep.
DVE_EVAC_OPS = ((0, 3), (1, 3), (2, 3))

# chunk start rows: disjoint except chunk 2, end-aligned to the 370-row
# shard (saves 14 zero-pad rows of HBM traffic). CLAIM = which shard rows
# each chunk OWNS for band construction (chunks 1/2 overlap in rows
# 242..256; chunk 1 owns them).
CH0 = (0, 128, SHR - 128)
CLAIM = ((0, 128), (128, 256), (256, SHR))


def _mm_parts(b):
    """[(tap, chunk)]: which chunks own rows that tap t of block b needs."""
    parts = []
    for t in range(KTAPS):
        lo, hi = 90 * b + t, 90 * b + t + M1  # shard rows [lo, hi)
        for c in range(NCH):
            clo, chi = CLAIM[c]
            if lo < chi and hi > clo:
                parts.append((t, c))
    return parts


def _build(nc, reps=1):
    xs = nc.declare_dram_parameter("xs", [SHR, ROW], BF, isOutput=False)
    nmm = sum(len(_mm_parts(b)) for b in range(NBLK))
    bd = nc.declare_dram_parameter("bands", [128, nmm, M1], BF, isOutput=False)
    am = nc.declare_dram_parameter("amc", [M1, 3, NOG, M2], BF, isOutput=False)
    # [blk, p, s, og, w]: one DMA per block; rows 120..128 junk
    out = nc.declare_dram_parameter("out", [NBLK, 128, 3, NOG, W], BF,
                                    isOutput=True)

    add = mybir.AluOpType.add

    with tile.TileContext(nc) as tc:
        with (
            tc.tile_pool(name="xa", bufs=2) as xpool,
            tc.tile_pool(name="tr", bufs=1) as tpool,
            tc.tile_pool(name="sp", bufs=3) as spool,
            tc.tile_pool(name="dd", bufs=4) as dpool,
            tc.tile_pool(name="ob", bufs=2) as opool,
            tc.tile_pool(name="cst", bufs=1) as cpool,
            tc.tile_pool(name="ps1", bufs=2, space="PSUM") as ps1pool,
            tc.tile_pool(name="ps2", bufs=2, space="PSUM") as ps2pool,
        ):
            bdt = cpool.tile([128, nmm, M1], BF)
            nc.sync.dma_start(bdt[:], bd[:])
            amt = cpool.tile([M1, 3, NOG, M2], BF)
            nc.sync.dma_start(amt[:], am[:])

            for it in range(reps):
                sps = []
                for ci in range(NCH):
                    xa = xpool.tile([128, ROW], BF, tag="xa", name=f"xa{ci}")
                    nc.sync.dma_start(xa[:], xs[CH0[ci]:CH0[ci] + 128])
                    xv = xa.rearrange("p (c w) -> p c w", c=C)
                    sp = spool.tile([128, SPW], BF, tag="sp", name=f"sp{ci}")
                    nc.vector.memset(sp[:, 0:WPAD], 0.0)
                    nc.vector.memset(sp[:, WPAD + W:SPW], 0.0)
                    # channel-sum tree in W-halves (bf16 2x DVE mode)
                    t1 = tpool.tile([128, 16, 360], BF, tag="t1", name="t1")
                    t2 = tpool.tile([128, 8, 360], BF, tag="t2", name="t2")
                    for h0 in (0, 360):
                        h1 = h0 + 360
                        nc.vector.tensor_tensor(out=t1[:],
                                                in0=xv[:, 0:16, h0:h1],
                                                in1=xv[:, 16:32, h0:h1],
                                                op=add)
                        nc.vector.tensor_tensor(out=t2[:], in0=t1[:, 0:8],
                                                in1=t1[:, 8:16], op=add)
                        t3 = t1[:, 0:4, :]
                        nc.vector.tensor_tensor(out=t3, in0=t2[:, 0:4],
                                                in1=t2[:, 4:8], op=add)
                        t4 = t2[:, 0:2, :]
                        nc.vector.tensor_tensor(out=t4, in0=t3[:, 0:2],
                                                in1=t3[:, 2:4], op=add)
                        nc.vector.tensor_tensor(
                            out=sp[:, WPAD + h0:WPAD + h1],
                            in0=t4[:, 0], in1=t4[:, 1], op=add)
                    sps.append(sp)

                # stage 1 for ALL blocks first: PE never stalls behind a
                # block's d-evac; d tiles (dd bufs=4) hold all four blocks
                dts = []
                mi = 0
                for blk in range(NBLK):
                    parts = _mm_parts(blk)
                    # 9-tap diagonal pool: accumulate over (tap, chunk) parts
                    dt = dpool.tile([M1, W], BF, tag="dt", name=f"dt{blk}")
                    for w0 in (0, 360):
                        ps1 = ps1pool.tile([M1, 360], FP, tag="ps1",
                                           name="ps1")
                        for pi, (t, ci) in enumerate(parts):
                            nc.tensor.matmul(
                                ps1[:], bdt[:, mi + pi, :],
                                sps[ci][:, w0 + t:w0 + t + 360],
                                start=(pi == 0), stop=(pi == len(parts) - 1),
                            )
                        nc.scalar.copy(out=dt[:, w0:w0 + 360], in_=ps1[:])
                    mi += len(parts)
                    dts.append(dt)

                for blk in range(NBLK):
                    dt = dts[blk]
                    # 3x1 conv + channel broadcast: strided-row packing,
                    # M = 4 channels x 30 rows; og pairs share a psum tile
                    ob3 = opool.tile([128, 3, NOG, W], BF, tag="ob",
                                     name="ob3")
                    for s in range(3):
                        for op in range(NOG // 2):
                            ps2 = ps2pool.tile([M2, 2 * W], FP, tag="ps2",
                                               name="ps2")
                            lhsA = amt[:, s, 2 * op, :]
                            lhsB = amt[:, s, 2 * op + 1, :]
                            nc.tensor.matmul(ps2[:, 0:512], lhsA,
                                             dt[0:M1, 0:512],
                                             start=True, stop=True)
                            nc.tensor.matmul(ps2[:, 512:720], lhsA,
                                             dt[0:M1, 512:W],
                                             start=True, stop=True)
                            nc.tensor.matmul(ps2[:, 720:1024], lhsB,
                                             dt[0:M1, 0:304],
                                             start=True, stop=True)
                            nc.tensor.matmul(ps2[:, 1024:1440], lhsB,
                                             dt[0:M1, 304:W],
                                             start=True, stop=True)
                            dst = ob3[0:M2, s, 2 * op:2 * op + 2, :]
                            src = ps2[:].rearrange("p (g w) -> p g w", g=2)
                            if (s, op) in DVE_EVAC_OPS:
                                nc.vector.tensor_copy(out=dst, in_=src)
                            else:
                                nc.scalar.copy(out=dst, in_=src)
                    nc.scalar.dma_start(out[blk], ob3[:])
    return nc


def _make_bands(half):
    """[128, nmm, 92] bf16 stage-1 bands, one [128, 92] slab per
    (block, tap, chunk) matmul, 1/9 scaled; d rows outside the global image
    zeroed (conv zero padding)."""
    nmm = sum(len(_mm_parts(b)) for b in range(NBLK))
    bands = np.zeros((128, nmm, M1), np.float32)
    mi = 0
    for b in range(NBLK):
        for (t, c) in _mm_parts(b):
            clo, chi = CLAIM[c]
            for m in range(M1):
                if half == 0 and b == 0 and m == 0:
                    continue  # d row -1
                if half == 1 and b == NBLK - 1 and m == M1 - 1:
                    continue  # d row 720
                r = 90 * b + m + t
                if clo <= r < chi:
                    bands[r - CH0[c], mi, m] = 1.0 / KTAPS
            mi += 1
    return bands.astype(NPBF)


def _make_amc(conv_w):
    """[92, 3, NOG, 120] bf16: 3-tap conv bands, strided-row packing.
    Sub-block s, slot oi*30+m -> out channel og*4+oi, block row 3m+s."""
    wsum = conv_w.sum(axis=1)[:, :, 0].astype(np.float64)  # [O, 3]
    amc = np.zeros((M1, 3, NOG, M2), np.float32)
    for s in range(3):
        for og in range(NOG):
            for oi in range(OSUB):
                o = og * OSUB + oi
                for m in range(NS):
                    for j in range(3):
                        amc[3 * m + s + j, s, og, oi * NS + m] = wsum[o, j]
    return amc.astype(NPBF)


def _make_shard(xt_b, h0):
    """xt_b: [H, ROW] bf16 one batch (h-major rows). [370, ROW] zero-pad."""
    sh = np.zeros((SHR, ROW), NPBF)
    lo, hi = h0 - HALO, h0 + HS + HALO
    slo, shi = max(lo, 0), min(hi, H)
    sh[slo - lo:shi - lo] = xt_b[slo:shi]
    return sh


def make_in_maps(x, conv_w):
    x = np.ascontiguousarray(np.asarray(x, dtype=np.float32))
    conv_w = np.asarray(conv_w, dtype=np.float32)
    assert x.shape == (B, C, H, W) and conv_w.shape == (O, C, 3, 1)
    xt = np.ascontiguousarray(x.transpose(0, 2, 1, 3)).astype(NPBF)
    xt = xt.reshape(B, H, ROW)
    amc = _make_amc(conv_w)
    bands = [_make_bands(0), _make_bands(1)]
    in_maps = []
    for i in range(N_CORES):
        b, half = i // 2, i % 2
        in_maps.append({
            "xs": _make_shard(xt[b], half * HS),
            "bands": bands[half],
            "amc": amc,
        })
    return in_maps


def assemble_out(results):
    out = np.empty((B, O, H, W), np.float32)
    for i in range(N_CORES):
        b, half = i // 2, i % 2
        v = np.asarray(results[i]["out"]).astype(np.float32)
        ov = np.empty((O, HS, W), np.float32)
        for blk in range(NBLK):
            for s in range(3):
                w = v[blk, 0:M2, s, :, :]             # [4*30, NOG, W]
                w = w.reshape(OSUB, NS, NOG, W)
                w = w.transpose(2, 0, 1, 3).reshape(O, NS, W)
                ov[:, blk * BLK + s:blk * BLK + 90:3, :] = w
        out[b, :, half * HS:(half + 1) * HS, :] = ov
    return out


def kernel(x, conv_w):
    nc = bacc.Bacc("TRN2", target_bir_lowering=False, debug=False,
                   num_devices=N_CORES)
    _build(nc)
    nc.compile()
    res = run_bass_kernel_spmd(nc, make_in_maps(x, conv_w),
                               list(range(N_CORES)), trace=False)
    return assemble_out(res.results)
